# revision 11
# baseline (speedup 1.0000x reference)
"""AttentionConv3D Trainium2 kernel.

Computation (per channel c, voxel (d,h,w)):
    q,k,v = 1x1x1 convs of x;  s_kv = q * (k_pad[nbr kv] + rel_bias(c,kv))
    out   = sum_kv softmax_kv(s) * v_pad[nbr kv]         (27 = 3x3x3 window)

Host<->device transfer over the axon tunnel (~50-90 MB/s) dominates wall
time, so the sharding/layout minimizes bytes moved:

H-shard over 8 cores: core i owns output rows 8i..8i+8 and receives the 10
padded H-rows 8i..8i+10 (1-row halo each side) of ALL 16 depth planes --
25% input overhead vs 100% for depth-sharding.  All traffic is fp16, packed
into one input tensor per core:
    cols [0, 16*10*WP)  x strip, n = d*(10*WP) + r*WP + wp  (WP = W+2 padded)
    then wk|wv|wq [64,64] each and rel-bias [64,27]
Output returns fp16 [64, 16*8*W] and is upcast on host.

On-device layout: partition p = channel (64), free dim = strip voxels.
K/V strips [18 planes, 10 rows, WP] f32/bf16 (depth-pad planes memset); the
1x1 convs project the already-zero-padded x so W/H pad cells come out zero,
matching the reference's pad-then-unfold semantics.  Per kv-neighbor the
window access is a free-dim offset (kd*660 + kh*66 + kw); the rel bias is a
per-partition scalar so s = (K_shift + B)*q is ONE DVE scalar_tensor_tensor
op.  exp on ACT (bias -28 keeps the table range; bf16 e/ev avoids fp16
underflow of exp(-28)); num/den accumulated with an on-device-built identity
matmul into PSUM; 1/den via exp(-ln(den)) on ACT.

The jitted PJRT executor is cached so repeat calls skip re-trace/re-jit, and
no zero output buffers are uploaded (the kernel writes every output element).
"""

import sys
import numpy as np

for _p in ("/opt/trn_rl_repo", "/root/.axon_site/_ro/trn_rl_repo"):
    if _p not in sys.path:
        sys.path.insert(0, _p)

NSPLIT = 1            # W-split pipelining factor (1 = single call)
D, H, W = 16, 64, 64
ROWS = 10             # strip rows per core: 8 output + 1 halo each side
_CACHE = {}


def _subs(L):
    return [(a, min(512, L - a)) for a in range(0, L, 512)]


def _build(wn):
    """Build the Bass program for output width wn (strip width wn+2)."""
    from contextlib import ExitStack
    import concourse.bacc as bacc
    import concourse.tile as tile
    from concourse import mybir

    wp = wn + 2                    # padded strip width
    pl = ROWS * wp                 # cols per (plane, strip): 10*wp
    xc = D * pl                    # x cols in the packed input
    on = 8 * wn                    # out cols per depth plane
    xcols = xc + 3 * 64 + 27

    f32 = mybir.dt.float32
    f16 = mybir.dt.float16
    bf16 = mybir.dt.bfloat16
    Alu = mybir.AluOpType
    Act = mybir.ActivationFunctionType

    nc = bacc.Bacc("TRN2", target_bir_lowering=False)
    xs_d = nc.dram_tensor("xs", [64, xcols], f16, kind="ExternalInput")
    out_d = nc.dram_tensor("out", [64, D * on], f16, kind="ExternalOutput")

    with tile.TileContext(nc) as tc, ExitStack() as ctx:
        singles = ctx.enter_context(tc.tile_pool(name="singles", bufs=1))
        planes = ctx.enter_context(tc.tile_pool(name="planes", bufs=1))
        wpool = ctx.enter_context(tc.tile_pool(name="work", bufs=2))

        Wt = singles.tile([64, 3 * 64 + 27], f16, tag="w")
        nc.sync.dma_start(Wt[:], xs_d[:, xc:xcols])
        wk_s = Wt[:, 0:64]
        wv_s = Wt[:, 64:128]
        wq_s = Wt[:, 128:192]
        b16 = Wt[:, 192:219]
        b_s = singles.tile([64, 27], f32, tag="b")
        nc.scalar.copy(b_s[:], b16)
        ebias = singles.tile([64, 1], f32, tag="ebias")
        nc.vector.memset(ebias[:], -28.0)
        id_s = singles.tile([64, 64], bf16, tag="id")
        nc.gpsimd.memset(id_s[:], 1.0)
        nc.gpsimd.affine_select(id_s[:], id_s[:], [[1, 64]], Alu.is_equal,
                                0.0, base=0, channel_multiplier=-1)

        # K/V strips: 18 depth planes (1 zero pad each side), 10 rows, wp cols
        Kt = planes.tile([64, (D + 2) * pl], f32, tag="k")
        Vt = planes.tile([64, (D + 2) * pl], bf16, tag="v")
        Q = planes.tile([64, D * on], f32, tag="q")
        OUT = planes.tile([64, D * on], f16, tag="o")
        nc.vector.memset(Kt[:, 0:pl], 0.0)
        nc.vector.memset(Kt[:, (D + 1) * pl:], 0.0)
        nc.gpsimd.memset(Vt[:, 0:pl], 0.0)
        nc.gpsimd.memset(Vt[:, (D + 1) * pl:], 0.0)

        X = planes.tile([64, xc], f16, tag="x")
        nc.sync.dma_start(X[:], xs_d[:, 0:xc])

        # ---- projections: one psum chunk per depth plane; the x strip is
        # already zero-padded so pad cells project to zero
        with tc.tile_pool(name="pp", bufs=2, space="PSUM") as ppool:
            for d in range(D):
                for w_s, kind in ((wk_s, "k"), (wv_s, "v"), (wq_s, "q")):
                    pp = ppool.tile([64, pl], f32, tag="pp")
                    for a, bl in _subs(pl):
                        nc.tensor.matmul(pp[:, a:a + bl], w_s,
                                         X[:, d * pl + a:d * pl + a + bl],
                                         start=True, stop=True)
                    dst = (d + 1) * pl
                    if kind == "k":
                        nc.vector.tensor_copy(Kt[:, dst:dst + pl], pp[:, :pl])
                    elif kind == "v":
                        nc.scalar.copy(Vt[:, dst:dst + pl], pp[:, :pl])
                    else:
                        # q: interior rows 1..8, cols 1..wn+1 only
                        nc.scalar.copy(
                            Q[:, d * on:(d + 1) * on].rearrange(
                                "p (r w) -> p r w", w=wn),
                            pp[:, :pl].rearrange(
                                "p (r w) -> p r w", w=wp)[:, 1:9, 1:wn + 1])

        # ---- 27-neighbor softmax attention, PSUM-chunked over depth planes
        accp = ctx.enter_context(tc.tile_pool(name="acc", bufs=1, space="PSUM"))
        Kv3 = Kt.rearrange("p (d r w) -> p d r w", r=ROWS, w=wp)
        Vv3 = Vt.rearrange("p (d r w) -> p d r w", r=ROWS, w=wp)
        GPSET = frozenset((0, 2, 6, 8, 9, 11, 15, 17, 18, 20, 21, 23, 24, 26))
        dchunks = [(d0, min(3, D - d0)) for d0 in range(0, D, 3)]
        for d0, nd in dchunks:
            L = nd * on
            den = accp.tile([64, 3 * 8 * 64], f32, tag="den")
            num = accp.tile([64, 3 * 8 * 64], f32, tag="num")
            for kv in range(27):
                kd, r = divmod(kv, 9)
                kh, kw = divmod(r, 3)
                # engine ops are limited to 3-D APs (partition + 2 free
                # dims), so depth planes get individual instructions
                s_t = wpool.tile([64, 3 * 8 * 64], f32, tag="s")
                for dl in range(nd):
                    nc.vector.scalar_tensor_tensor(
                        s_t[:, dl * on:(dl + 1) * on].rearrange(
                            "p (r w) -> p r w", w=wn),
                        Kv3[:, d0 + kd + dl, kh:kh + 8, kw:kw + wn],
                        b_s[:, kv:kv + 1],
                        Q[:, (d0 + dl) * on:(d0 + dl + 1) * on].rearrange(
                            "p (r w) -> p r w", w=wn),
                        Alu.add, Alu.mult)
                e_t = wpool.tile([64, 3 * 8 * 64], bf16, tag="e")
                # bias keeps exp inside the ACT table range (softmax is
                # shift-invariant; the -28 cancels via the ln/exp normalize)
                nc.scalar.activation(e_t[:, :L], s_t[:, :L], Act.Exp,
                                     bias=ebias[:])
                ev_t = wpool.tile([64, 3 * 8 * 64], bf16, tag="ev")
                # split e*v products between DVE and the otherwise-idle GPSIMD
                ev_eng = nc.gpsimd if (kw == 1 or kv in GPSET) else nc.vector
                for dl in range(nd):
                    ev_eng.tensor_mul(
                        ev_t[:, dl * on:(dl + 1) * on].rearrange(
                            "p (r w) -> p r w", w=wn),
                        e_t[:, dl * on:(dl + 1) * on].rearrange(
                            "p (r w) -> p r w", w=wn),
                        Vv3[:, d0 + kd + dl, kh:kh + 8, kw:kw + wn])
                st, sp = kv == 0, kv == 26
                for a, bl in _subs(L):
                    nc.tensor.matmul(den[:, a:a + bl], id_s[:],
                                     e_t[:, a:a + bl], start=st, stop=sp)
                    nc.tensor.matmul(num[:, a:a + bl], id_s[:],
                                     ev_t[:, a:a + bl], start=st, stop=sp)
            l_t = wpool.tile([64, 3 * 8 * 64], f32, tag="s")
            nc.scalar.activation(l_t[:, :L], den[:, :L], Act.Ln)
            f_t = wpool.tile([64, 3 * 8 * 64], f32, tag="f")
            nc.scalar.activation(f_t[:, :L], l_t[:, :L], Act.Exp, scale=-1.0)
            nc.vector.tensor_mul(OUT[:, d0 * on:d0 * on + L],
                                 num[:, :L], f_t[:, :L])
            nc.sync.dma_start(out_d[:, d0 * on:d0 * on + L],
                              OUT[:, d0 * on:d0 * on + L])
    nc.finalize()
    return nc


def _make_runner(wn):
    import jax
    from jax.sharding import Mesh, PartitionSpec
    from jax.experimental.shard_map import shard_map
    from concourse import mybir
    from concourse.bass2jax import (
        install_neuronx_cc_hook, partition_id_tensor, _bass_exec_p)

    nc = _build(wn)
    install_neuronx_cc_hook()
    partition_name = (nc.partition_id_tensor.name
                      if nc.partition_id_tensor else None)
    in_names, out_names, out_avals = [], [], []
    for alloc in nc.m.functions[0].allocations:
        if not isinstance(alloc, mybir.MemoryLocationSet):
            continue
        name = alloc.memorylocations[0].name
        if alloc.kind == "ExternalInput":
            if name != partition_name:
                in_names.append(name)
        elif alloc.kind == "ExternalOutput":
            out_names.append(name)
            out_avals.append(jax.core.ShapedArray(
                tuple(alloc.tensor_shape), mybir.dt.np(alloc.dtype)))
    # out-named operands are omitted: the kernel writes every output element,
    # so no pre-zeroed donated buffers are needed (saves their host upload)
    all_names = tuple(in_names)
    if partition_name is not None:
        all_names = all_names + (partition_name,)

    def _body(*args):
        operands = list(args)
        if partition_name is not None:
            operands.append(partition_id_tensor())
        outs = _bass_exec_p.bind(
            *operands, out_avals=tuple(out_avals), in_names=all_names,
            out_names=tuple(out_names), lowering_input_output_aliases=(),
            sim_require_finite=True, sim_require_nnan=True, nc=nc)
        return tuple(outs)

    n_cores = 8
    devices = jax.devices()[:n_cores]
    mesh = Mesh(np.asarray(devices), ("core",))
    sharded = jax.jit(
        shard_map(_body, mesh=mesh,
                  in_specs=(PartitionSpec("core"),) * len(in_names),
                  out_specs=(PartitionSpec("core"),) * len(out_names),
                  check_rep=False),
        keep_unused=True)
    return sharded


def kernel(x, w_q, w_k, w_v, rel_d, rel_h, rel_w):
    x = np.asarray(x, np.float32)
    rd = np.asarray(rel_d, np.float32).reshape(21, 3)
    rh = np.asarray(rel_h, np.float32).reshape(21, 3)
    rw = np.asarray(rel_w, np.float32).reshape(22, 3)

    wn = W // NSPLIT
    wp = wn + 2
    pl = ROWS * wp
    xc = D * pl
    xcols = xc + 3 * 64 + 27
    on = 8 * wn

    # rel bias table: rows = channel, cols = kv = kd*9+kh*3+kw
    kvi = np.arange(27)
    wpack = np.empty((64, 3 * 64 + 27), np.float16)
    wpack[:, 0:64] = w_k.T
    wpack[:, 64:128] = w_v.T
    wpack[:, 128:192] = w_q.T
    Bh = np.empty((64, 27), np.float16)
    Bh[0:21] = rd[:, kvi // 9]
    Bh[21:42] = rh[:, (kvi % 9) // 3]
    Bh[42:64] = rw[:, kvi % 3]
    wpack[:, 192:219] = Bh

    # globally padded x: [c, d, 66 rows, 66 cols]
    xr = np.zeros((64, D, H + 2, W + 2), np.float16)
    xr[:, :, 1:65, 1:65] = x[0]

    if "runs" not in _CACHE:
        _CACHE["runs"] = _make_runner(wn)

    outs = []
    for j in range(NSPLIT):
        xs_all = np.empty((8 * 64, xcols), np.float16)
        for i in range(8):
            xs_all[64 * i:64 * i + 64, :xc] = \
                xr[:, :, 8 * i:8 * i + ROWS,
                   j * wn:j * wn + wp].reshape(64, xc)
            xs_all[64 * i:64 * i + 64, xc:] = wpack
        outs.append(_CACHE["runs"](xs_all))

    full = np.empty((64, D, H, W), np.float32)
    for j in range(NSPLIT):
        res = np.asarray(outs[j][0])       # [8*64, D*8*wn] fp16
        res = res.reshape(8, 64, D, 8, wn)
        # out[c, d, 8i+r, j*wn + w] = res[i, c, d, r, w]
        full[:, :, :, j * wn:(j + 1) * wn] = \
            res.transpose(1, 2, 0, 3, 4).reshape(64, D, H, wn)
    return np.ascontiguousarray(full.reshape(1, 64, D, H, W))


# revision 12
# speedup vs baseline: 2.4261x; 2.4261x over previous
"""AttentionConv3D Trainium2 kernel.

Computation (per channel c, voxel (d,h,w)):
    q,k,v = 1x1x1 convs of x;  s_kv = q * (k_pad[nbr kv] + rel_bias(c,kv))
    out   = sum_kv softmax_kv(s) * v_pad[nbr kv]         (27 = 3x3x3 window)

Host<->device transfer over the axon tunnel (~50-90 MB/s) dominates wall
time, so the sharding/layout minimizes bytes moved:

H-shard over 8 cores: core i owns output rows 8i..8i+8 and receives the 10
padded H-rows 8i..8i+10 (1-row halo each side) of ALL 16 depth planes --
25% input overhead vs 100% for depth-sharding.  All traffic is fp16, packed
into one input tensor per core:
    cols [0, 16*10*WP)  x strip, n = d*(10*WP) + r*WP + wp  (WP = W+2 padded)
    then wk|wv|wq [64,64] each and rel-bias [64,27]
Output returns fp16 [64, 16*8*W] and is upcast on host.

On-device layout: partition p = channel (64), free dim = strip voxels.
K/V strips [18 planes, 10 rows, WP] f32/bf16 (depth-pad planes memset); the
1x1 convs project the already-zero-padded x so W/H pad cells come out zero,
matching the reference's pad-then-unfold semantics.  Per kv-neighbor the
window access is a free-dim offset (kd*660 + kh*66 + kw); the rel bias is a
per-partition scalar so s = (K_shift + B)*q is ONE DVE scalar_tensor_tensor
op.  exp on ACT (bias -28 keeps the table range; bf16 e/ev avoids fp16
underflow of exp(-28)); num/den accumulated with an on-device-built identity
matmul into PSUM; 1/den via exp(-ln(den)) on ACT.

The jitted PJRT executor is cached so repeat calls skip re-trace/re-jit, and
no zero output buffers are uploaded (the kernel writes every output element).
"""

import sys
import numpy as np

for _p in ("/opt/trn_rl_repo", "/root/.axon_site/_ro/trn_rl_repo"):
    if _p not in sys.path:
        sys.path.insert(0, _p)

NSPLIT = 1            # W-split pipelining factor (1 = single call)
D, H, W = 16, 64, 64
ROWS = 10             # strip rows per core: 8 output + 1 halo each side
_CACHE = {}


def _subs(L):
    return [(a, min(512, L - a)) for a in range(0, L, 512)]


def _build(wn):
    """Build the Bass program for output width wn (strip width wn+2)."""
    from contextlib import ExitStack
    import concourse.bacc as bacc
    import concourse.tile as tile
    from concourse import mybir

    wp = wn + 2                    # padded strip width
    pl = ROWS * wp                 # cols per (plane, strip): 10*wp
    xc = D * pl                    # x cols in the packed input
    on = 8 * wn                    # out cols per depth plane
    xcols = xc + 3 * 64 + 27

    f32 = mybir.dt.float32
    f16 = mybir.dt.float16
    bf16 = mybir.dt.bfloat16
    Alu = mybir.AluOpType
    Act = mybir.ActivationFunctionType

    nc = bacc.Bacc("TRN2", target_bir_lowering=False)
    xs_d = nc.dram_tensor("xs", [64, xcols], f16, kind="ExternalInput")
    out_d = nc.dram_tensor("out", [64, D * on], f16, kind="ExternalOutput")

    with tile.TileContext(nc) as tc, ExitStack() as ctx:
        singles = ctx.enter_context(tc.tile_pool(name="singles", bufs=1))
        planes = ctx.enter_context(tc.tile_pool(name="planes", bufs=1))
        wpool = ctx.enter_context(tc.tile_pool(name="work", bufs=2))

        Wt = singles.tile([64, 3 * 64 + 27], f16, tag="w")
        nc.sync.dma_start(Wt[:], xs_d[:, xc:xcols])
        wk_s = Wt[:, 0:64]
        wv_s = Wt[:, 64:128]
        wq_s = Wt[:, 128:192]
        b16 = Wt[:, 192:219]
        b_s = singles.tile([64, 27], f32, tag="b")
        nc.scalar.copy(b_s[:], b16)
        ebias = singles.tile([64, 1], f32, tag="ebias")
        nc.vector.memset(ebias[:], -28.0)
        id_s = singles.tile([64, 64], bf16, tag="id")
        nc.gpsimd.memset(id_s[:], 1.0)
        nc.gpsimd.affine_select(id_s[:], id_s[:], [[1, 64]], Alu.is_equal,
                                0.0, base=0, channel_multiplier=-1)

        # K/V strips: 18 depth planes (1 zero pad each side), 10 rows, wp cols
        Kt = planes.tile([64, (D + 2) * pl], f32, tag="k")
        Vt = planes.tile([64, (D + 2) * pl], bf16, tag="v")
        Q = planes.tile([64, D * on], f32, tag="q")
        OUT = planes.tile([64, D * on], f16, tag="o")
        nc.vector.memset(Kt[:, 0:pl], 0.0)
        nc.vector.memset(Kt[:, (D + 1) * pl:], 0.0)
        nc.gpsimd.memset(Vt[:, 0:pl], 0.0)
        nc.gpsimd.memset(Vt[:, (D + 1) * pl:], 0.0)

        X = planes.tile([64, xc], f16, tag="x")
        nc.sync.dma_start(X[:], xs_d[:, 0:xc])

        # ---- projections: one psum chunk per depth plane; the x strip is
        # already zero-padded so pad cells project to zero
        with tc.tile_pool(name="pp", bufs=2, space="PSUM") as ppool:
            for d in range(D):
                for w_s, kind in ((wk_s, "k"), (wv_s, "v"), (wq_s, "q")):
                    pp = ppool.tile([64, pl], f32, tag="pp")
                    for a, bl in _subs(pl):
                        nc.tensor.matmul(pp[:, a:a + bl], w_s,
                                         X[:, d * pl + a:d * pl + a + bl],
                                         start=True, stop=True)
                    dst = (d + 1) * pl
                    if kind == "k":
                        nc.vector.tensor_copy(Kt[:, dst:dst + pl], pp[:, :pl])
                    elif kind == "v":
                        nc.scalar.copy(Vt[:, dst:dst + pl], pp[:, :pl])
                    else:
                        # q: interior rows 1..8, cols 1..wn+1 only
                        nc.scalar.copy(
                            Q[:, d * on:(d + 1) * on].rearrange(
                                "p (r w) -> p r w", w=wn),
                            pp[:, :pl].rearrange(
                                "p (r w) -> p r w", w=wp)[:, 1:9, 1:wn + 1])

        # ---- 27-neighbor softmax attention, PSUM-chunked over depth planes
        accp = ctx.enter_context(tc.tile_pool(name="acc", bufs=1, space="PSUM"))
        Kv3 = Kt.rearrange("p (d r w) -> p d r w", r=ROWS, w=wp)
        Vv3 = Vt.rearrange("p (d r w) -> p d r w", r=ROWS, w=wp)
        GPSET = frozenset((0, 2, 6, 8, 9, 11, 15, 17, 18, 20, 21, 23, 24, 26))
        dchunks = [(d0, min(3, D - d0)) for d0 in range(0, D, 3)]
        for d0, nd in dchunks:
            L = nd * on
            den = accp.tile([64, 3 * 8 * 64], f32, tag="den")
            num = accp.tile([64, 3 * 8 * 64], f32, tag="num")
            for kv in range(27):
                kd, r = divmod(kv, 9)
                kh, kw = divmod(r, 3)
                # engine ops are limited to 3-D APs (partition + 2 free
                # dims), so depth planes get individual instructions
                s_t = wpool.tile([64, 3 * 8 * 64], f32, tag="s")
                for dl in range(nd):
                    nc.vector.scalar_tensor_tensor(
                        s_t[:, dl * on:(dl + 1) * on].rearrange(
                            "p (r w) -> p r w", w=wn),
                        Kv3[:, d0 + kd + dl, kh:kh + 8, kw:kw + wn],
                        b_s[:, kv:kv + 1],
                        Q[:, (d0 + dl) * on:(d0 + dl + 1) * on].rearrange(
                            "p (r w) -> p r w", w=wn),
                        Alu.add, Alu.mult)
                e_t = wpool.tile([64, 3 * 8 * 64], bf16, tag="e")
                # bias keeps exp inside the ACT table range (softmax is
                # shift-invariant; the -28 cancels via the ln/exp normalize)
                nc.scalar.activation(e_t[:, :L], s_t[:, :L], Act.Exp,
                                     bias=ebias[:])
                ev_t = wpool.tile([64, 3 * 8 * 64], bf16, tag="ev")
                # split e*v products between DVE and the otherwise-idle GPSIMD
                ev_eng = nc.gpsimd if (kw == 1 or kv in GPSET) else nc.vector
                for dl in range(nd):
                    ev_eng.tensor_mul(
                        ev_t[:, dl * on:(dl + 1) * on].rearrange(
                            "p (r w) -> p r w", w=wn),
                        e_t[:, dl * on:(dl + 1) * on].rearrange(
                            "p (r w) -> p r w", w=wn),
                        Vv3[:, d0 + kd + dl, kh:kh + 8, kw:kw + wn])
                st, sp = kv == 0, kv == 26
                for a, bl in _subs(L):
                    nc.tensor.matmul(den[:, a:a + bl], id_s[:],
                                     e_t[:, a:a + bl], start=st, stop=sp)
                    nc.tensor.matmul(num[:, a:a + bl], id_s[:],
                                     ev_t[:, a:a + bl], start=st, stop=sp)
            l_t = wpool.tile([64, 3 * 8 * 64], f32, tag="s")
            nc.scalar.activation(l_t[:, :L], den[:, :L], Act.Ln)
            f_t = wpool.tile([64, 3 * 8 * 64], f32, tag="f")
            nc.scalar.activation(f_t[:, :L], l_t[:, :L], Act.Exp, scale=-1.0)
            nc.vector.tensor_mul(OUT[:, d0 * on:d0 * on + L],
                                 num[:, :L], f_t[:, :L])
            nc.sync.dma_start(out_d[:, d0 * on:d0 * on + L],
                              OUT[:, d0 * on:d0 * on + L])
    nc.finalize()
    return nc


def _make_runner(wn):
    import jax
    from jax.sharding import Mesh, PartitionSpec
    from jax.experimental.shard_map import shard_map
    from concourse import mybir
    from concourse.bass2jax import (
        install_neuronx_cc_hook, partition_id_tensor, _bass_exec_p)

    nc = _build(wn)
    install_neuronx_cc_hook()
    partition_name = (nc.partition_id_tensor.name
                      if nc.partition_id_tensor else None)
    in_names, out_names, out_avals = [], [], []
    for alloc in nc.m.functions[0].allocations:
        if not isinstance(alloc, mybir.MemoryLocationSet):
            continue
        name = alloc.memorylocations[0].name
        if alloc.kind == "ExternalInput":
            if name != partition_name:
                in_names.append(name)
        elif alloc.kind == "ExternalOutput":
            out_names.append(name)
            out_avals.append(jax.core.ShapedArray(
                tuple(alloc.tensor_shape), mybir.dt.np(alloc.dtype)))
    # out-named operands are omitted: the kernel writes every output element,
    # so no pre-zeroed donated buffers are needed (saves their host upload)
    all_names = tuple(in_names)
    if partition_name is not None:
        all_names = all_names + (partition_name,)

    def _body(*args):
        operands = list(args)
        if partition_name is not None:
            operands.append(partition_id_tensor())
        outs = _bass_exec_p.bind(
            *operands, out_avals=tuple(out_avals), in_names=all_names,
            out_names=tuple(out_names), lowering_input_output_aliases=(),
            sim_require_finite=True, sim_require_nnan=True, nc=nc)
        return tuple(outs)

    n_cores = 8
    devices = jax.devices()[:n_cores]
    mesh = Mesh(np.asarray(devices), ("core",))
    sharded = jax.jit(
        shard_map(_body, mesh=mesh,
                  in_specs=(PartitionSpec("core"),) * len(in_names),
                  out_specs=(PartitionSpec("core"),) * len(out_names),
                  check_rep=False),
        keep_unused=True)
    return sharded


def kernel(x, w_q, w_k, w_v, rel_d, rel_h, rel_w):
    x = np.asarray(x, np.float32)
    rd = np.asarray(rel_d, np.float32).reshape(21, 3)
    rh = np.asarray(rel_h, np.float32).reshape(21, 3)
    rw = np.asarray(rel_w, np.float32).reshape(22, 3)

    wn = W // NSPLIT
    wp = wn + 2
    pl = ROWS * wp
    xc = D * pl
    xcols = xc + 3 * 64 + 27
    on = 8 * wn

    # rel bias table: rows = channel, cols = kv = kd*9+kh*3+kw
    kvi = np.arange(27)
    wpack = np.empty((64, 3 * 64 + 27), np.float16)
    wpack[:, 0:64] = w_k.T
    wpack[:, 64:128] = w_v.T
    wpack[:, 128:192] = w_q.T
    Bh = np.empty((64, 27), np.float16)
    Bh[0:21] = rd[:, kvi // 9]
    Bh[21:42] = rh[:, (kvi % 9) // 3]
    Bh[42:64] = rw[:, kvi % 3]
    wpack[:, 192:219] = Bh

    # globally padded x: [c, d, 66 rows, 66 cols]; pad cells stay zero across
    # calls, only the interior is rewritten
    if "xr" not in _CACHE:
        _CACHE["xr"] = np.zeros((64, D, H + 2, W + 2), np.float16)
    xr = _CACHE["xr"]
    xr[:, :, 1:65, 1:65] = x[0]

    if "runs" not in _CACHE:
        _CACHE["runs"] = _make_runner(wn)

    outs = []
    for j in range(NSPLIT):
        xs_all = np.empty((8 * 64, xcols), np.float16)
        for i in range(8):
            xs_all[64 * i:64 * i + 64, :xc] = \
                xr[:, :, 8 * i:8 * i + ROWS,
                   j * wn:j * wn + wp].reshape(64, xc)
            xs_all[64 * i:64 * i + 64, xc:] = wpack
        outs.append(_CACHE["runs"](xs_all))

    # fetch output shards concurrently; each shard i is core i's H-band, so
    # the scatter into `full` needs no transpose
    import threading
    full = np.empty((64, D, H, W), np.float32)

    def fetch(j, shard):
        i = shard.index[0].start // 64
        res = np.asarray(shard.data).reshape(64, D, 8, wn)
        full[:, :, 8 * i:8 * i + 8, j * wn:(j + 1) * wn] = res

    for j in range(NSPLIT):
        shards = list(outs[j][0].addressable_shards)
        ths = [threading.Thread(target=fetch, args=(j, s)) for s in shards[1:]]
        for t in ths:
            t.start()
        fetch(j, shards[0])
        for t in ths:
            t.join()
    return full.reshape(1, 64, D, H, W)
